# revision 18
# baseline (speedup 1.0000x reference)
"""AttentionalGNN Trainium2 kernel — 8-core SPMD.

Sharding: core c = (b, q) with b = c // 4 (batch), q = c % 4 (node quarter,
256 of 1024 nodes). Every core runs an identical program; per-core behavior
differs only through input data (its batch's desc tensors and its node
slice). Per layer:
  - k/v^T convs computed on the full node axis (replicated within a batch
    group, from bf16 slabs), q/MLP/attention computed for the local node
    quarter,
  - BatchNorm statistics exchanged via an 8-way AllGather + local sum-tree
    (cheaper than AllReduce in latency), one per stream so each overlaps
    the other stream's attention+MLP,
  - layer outputs AllGather'd in bf16 within each batch group of 4, one
    collective per stream, staggered so the next layer's first attention
    unit starts as soon as its source slab lands.
Matmuls on the fp32 path run as float32r (full-rate fp32); k/v convs run in
bf16 (inputs are bf16 gathered slabs). Softmax uses no max-subtraction
(|scores| <= ~64 for this model, exp stays in fp32 range); the per-node
softmax denominator comes from a ones-column folded into v^T, and the
division is folded into the PSUM->SBUF evacuation of the message matmul.
All DRAM layouts are partition-major so every DMA is per-partition
contiguous.

The compiled executable (jax.jit of a shard_map'd bass_exec call) is cached
in-process so repeated kernel() calls skip retracing and recompilation.
"""

import numpy as np
import ml_dtypes

import concourse.bass as bass
import concourse.tile as tile
from concourse import bacc, bass2jax, mybir

L, D, H, B, N = 18, 256, 4, 2, 1024
HD = D // H           # 64
NL = N // 4           # 256 local nodes per core
EPS = 1e-5
F32 = mybir.dt.float32
F32R = mybir.dt.float32r
BF16 = mybir.dt.bfloat16
I32 = mybir.dt.int32
AF = mybir.ActivationFunctionType
OP = mybir.AluOpType
NP_BF16 = ml_dtypes.bfloat16

# head-contiguous channel permutation: perm[h*64+hd] = hd*4+h
PERM = np.array([hd * H + h for h in range(H) for hd in range(HD)], np.int64)

_CACHE = {}


def _r(ap):
    return ap.bitcast(F32R)


def _unit_order(li):
    # process first the unit whose source slab arrives first (see AG stagger)
    return (0, 1) if li % 4 in (0, 3) else (1, 0)


def _build_program(n_layers=L, use_coll=True, num_devices=8,
                   bnA_pos="after_attnB", merge_stats=True, defer=False):
    nc = bacc.Bacc("TRN2", target_bir_lowering=False, debug=False,
                   num_devices=num_devices)

    dram = {}
    def din(name, shape, dt=F32):
        dram[name] = nc.dram_tensor(name, shape, dt, kind="ExternalInput")
    din("wqm", [L, 128, 2, 2, 256])        # (p, k, {q,m}, o) fp32
    din("wkv", [L, 128, 2, 2, 256], BF16)  # (p, k, {k,v}, o) bf16
    din("w1t", [L, 128, 4, 512])
    din("w2t", [L, 128, 4, 256])
    din("bia", [L, 128, 26])               # bq2 bm2 b1(4) b2(2) g1(8) be1(8)
    din("dsc", [128, 2, 2, N], BF16)       # (p, chunk, stream, n) full descs
    din("dlo", [128, 2, 2, NL])            # (p, stream, chunk, n) local slices
    out_d = nc.dram_tensor("out", [L, 128, 2, 2, NL], F32, kind="ExternalOutput")

    RG_ALL = [list(range(8))]
    RG_B = [[0, 1, 2, 3], [4, 5, 6, 7]]

    with tile.TileContext(nc) as tc:
        from contextlib import ExitStack
        _es = ExitStack()
        wp = _es.enter_context(tc.tile_pool(name="wp", bufs=2))
        a2 = _es.enter_context(tc.tile_pool(name="a2", bufs=2))
        a1 = _es.enter_context(tc.tile_pool(name="a1", bufs=1))
        ep = _es.enter_context(tc.tile_pool(name="ep", bufs=4))
        p4 = _es.enter_context(tc.tile_pool(name="p4", bufs=2, space="PSUM"))
        p256 = _es.enter_context(tc.tile_pool(name="p256", bufs=2, space="PSUM"))
        pmsg = _es.enter_context(tc.tile_pool(name="pmsg", bufs=2, space="PSUM"))
        dp = _es.enter_context(tc.tile_pool(name="dp", bufs=2, space="DRAM"))

        # ---- persistent tiles ----
        slab_t = [[a1.tile([128, 2, N], BF16, tag=f"sl{s}{pp}", name=f"sl{s}{pp}")
                   for pp in range(2)] for s in range(2)]
        for s in range(2):
            nc.sync.dma_start(out=slab_t[s][0][:],
                              in_=dram["dsc"].ap()[:, :, s, :])
        slabs = [slab_t[0][0], slab_t[1][0]]
        dl = a1.tile([128, 2, 2, NL], F32, tag="dl", name="dl")
        nc.sync.dma_start(out=_r(dl[:]), in_=_r(dram["dlo"].ap()))
        # vT tiles with persistent ones columns, double-buffered by parity
        vt_t = [[[a1.tile([128, 260], F32, tag=f"v{u}{f}{pp}", name=f"v{u}{f}{pp}")
                  for f in range(8)] for u in range(2)] for pp in range(2)]
        for pp in range(2):
            for u in range(2):
                for f in range(8):
                    tv = vt_t[pp][u][f][:].rearrange("p (h c) -> p h c", h=4)
                    nc.vector.memset(tv[:, :, 64:65], 1.0)

        # ---- deferred per-stream BN/conv2/gather (software pipelined across
        # layers: stream A's tail is emitted mid-layer behind stream B's
        # attention; stream B's tail is deferred into the next layer, hidden
        # behind its first k/v convs) ----
        state = {"pending": None, "xprev": None}

        def bn_conv2(cx):
            nc_ = nc
            u, li = cx["u"], cx["li"]
            stg_u, ht_u = cx["stg"], cx["ht"]
            xn, xn16 = cx["xn"], cx["xn16"]
            g1u = cx["g1"][:, u * 4:(u + 1) * 4]
            be1u = cx["be1"][:, u * 4:(u + 1) * 4]
            mean_t = a2.tile([128, 4], F32, tag=f"mean{u}", name=f"mean{li}{u}")
            var_t = a2.tile([128, 4], F32, tag=f"var{u}", name=f"var{li}{u}")
            sc_t = a2.tile([128, 4], F32, tag=f"scl{u}", name=f"scl{li}{u}")
            sh_t = a2.tile([128, 4], F32, tag=f"shf{u}", name=f"shf{li}{u}")
            nc_.vector.tensor_scalar(mean_t[:], stg_u[:, 0, :], 1.0 / 2048.0,
                                     None, OP.mult)
            nc_.vector.tensor_scalar(var_t[:], stg_u[:, 1, :], 1.0 / 2048.0,
                                     None, OP.mult)
            nc_.vector.tensor_tensor(sc_t[:], mean_t[:], mean_t[:], OP.mult)
            nc_.vector.tensor_tensor(var_t[:], var_t[:], sc_t[:], OP.subtract)
            nc_.vector.tensor_scalar(var_t[:], var_t[:], EPS, None, OP.add)
            # rsqrt via magic-constant seed + 2 Newton steps (DVE only,
            # avoids ACT Ln/Sqrt which would force activation-table swaps)
            y_t = a2.tile([128, 4], F32, tag=f"rsq{u}", name=f"rsq{li}{u}")
            t_t = a2.tile([128, 4], F32, tag=f"rst{u}", name=f"rst{li}{u}")
            nc_.vector.tensor_scalar(y_t[:].bitcast(I32), var_t[:].bitcast(I32),
                                     1, None, OP.logical_shift_right)
            nc_.vector.tensor_scalar(y_t[:].bitcast(I32), y_t[:].bitcast(I32),
                                     -1, 0x5f3759df, OP.mult, OP.add)
            for _newton in range(2):
                nc_.vector.tensor_tensor(t_t[:], y_t[:], y_t[:], OP.mult)
                nc_.vector.tensor_tensor(t_t[:], t_t[:], var_t[:], OP.mult)
                nc_.vector.tensor_scalar(t_t[:], t_t[:], -0.5, 1.5, OP.mult, OP.add)
                nc_.vector.tensor_tensor(y_t[:], y_t[:], t_t[:], OP.mult)
            nc_.vector.tensor_tensor(sc_t[:], y_t[:], g1u, OP.mult)
            nc_.vector.tensor_tensor(sh_t[:], mean_t[:], sc_t[:], OP.mult)
            nc_.vector.tensor_tensor(sh_t[:], be1u, sh_t[:], OP.subtract)
            hn = a1.tile([128, 4, NL], F32, tag=f"hn{u}", name=f"hn{li}{u}")
            for mo in range(4):
                nc_.scalar.activation(_r(hn[:, mo, :]), ht_u[:, mo, :], AF.Relu,
                                      bias=sh_t[:, mo:mo + 1],
                                      scale=sc_t[:, mo:mo + 1])
            for mo in range(2):
                ps = p256.tile([128, NL], F32, tag="p256", name=f"o2p{li}{u}{mo}")
                for k in range(4):
                    nc_.tensor.matmul(ps[:],
                                      _r(cx["w2"][:, k, mo * 128:(mo + 1) * 128]),
                                      _r(hn[:, k, :]), start=(k == 0), stop=(k == 3))
                nc_.vector.tensor_scalar(_r(xn[:, u, mo, :]), ps[:],
                                         cx["b2"][:, mo:mo + 1], None, OP.add)
            nc_.vector.tensor_tensor(_r(xn[:, u]), xn[:, u], cx["resid"][:, u],
                                     OP.add)
            nc_.vector.tensor_copy(xn16[:, u], xn[:, u])

        def gather(cx):
            u, li, npar = cx["u"], cx["li"], cx["npar"]
            agi = dp.tile([128, 2, NL], BF16, tag=f"agi{u}", name=f"agi{li}{u}")
            ago = dp.tile([4, 128, 2, NL], BF16, tag=f"ago{u}", name=f"ago{li}{u}")
            nc.gpsimd.dma_start(out=agi[:], in_=cx["xn16"][:, u])
            if use_coll:
                nc.gpsimd.collective_compute("AllGather", OP.bypass,
                                             replica_groups=RG_B,
                                             ins=[agi[:].opt()],
                                             outs=[ago[:].opt()])
            else:
                # timing-only fallback: satisfy the dependency with one
                # local DMA (values in groups 1..3 are garbage)
                nc.sync.dma_start(out=ago[0], in_=agi[:])
            t = slab_t[u][npar]
            for c in range(2):
                nc.sync.dma_start(
                    out=t[:, c, :].rearrange("p (q n) -> p q n", q=4),
                    in_=ago[:, :, c, :].rearrange("q p n -> p q n"))
            slabs[u] = t

        def flush_pending():
            cx = state["pending"]
            if cx is None:
                return
            state["pending"] = None
            bn_conv2(cx)
            if not cx["last"]:
                gather(cx)
            nc.gpsimd.dma_start(out=out_d.ap()[cx["i"]], in_=cx["xn"][:])

        for li in range(n_layers):
            i = li % L
            par = li % 2
            npar = (li + 1) % 2
            w4_t = wp.tile([128, 2, 2, 256], F32, tag="w4", name=f"w4_{li}")
            nc.sync.dma_start(out=_r(w4_t[:]), in_=_r(dram["wqm"].ap()[i]))
            wkv_t = wp.tile([128, 2, 2, 256], BF16, tag="wkv", name=f"wkv_{li}")
            nc.sync.dma_start(out=wkv_t[:], in_=dram["wkv"].ap()[i])
            w1_t = wp.tile([128, 4, 512], F32, tag="w1", name=f"w1_{li}")
            nc.sync.dma_start(out=_r(w1_t[:]), in_=_r(dram["w1t"].ap()[i]))
            w2_t = wp.tile([128, 4, 256], F32, tag="w2", name=f"w2_{li}")
            nc.sync.dma_start(out=_r(w2_t[:]), in_=_r(dram["w2t"].ap()[i]))
            bia_t = wp.tile([128, 26], F32, tag="bia", name=f"bia_{li}")
            nc.sync.dma_start(out=bia_t[:], in_=dram["bia"].ap()[i])
            wq_t = w4_t[:, :, 0, :]; wm_t = w4_t[:, :, 1, :]
            wk_t = wkv_t[:, :, 0, :]; wv_t = wkv_t[:, :, 1, :]
            bq_c = bia_t[:, 0:2]; bm_c = bia_t[:, 2:4]
            b1_c = bia_t[:, 4:8]; b2_c = bia_t[:, 8:10]
            g1_c = bia_t[:, 10:18]; be1_c = bia_t[:, 18:26]

            self_layer = (li % 2 == 0)
            order = _unit_order(li)
            uA, uB = order
            sA = uA if self_layer else 1 - uA
            sB = uB if self_layer else 1 - uB
            xcur = state["xprev"] if li > 0 else dl
            resid = dl if li <= 1 else state["xprev"]

            xn = a2.tile([128, 2, 2, NL], F32, tag="xn", name=f"xn{li}")
            xn16 = a2.tile([128, 2, 2, NL], BF16, tag="xn16", name=f"xn16{li}")
            qt = [None, None]
            kt = [None, None]
            vt = vt_t[par]
            msgt = [None, None]
            stg = [None, None]
            stl_d = [None, None]
            ht = [None, None]

            def q_unit(u):
                qtile = a2.tile([128, 2, NL], F32, tag=f"q{u}", name=f"q{li}{u}")
                for mo in range(2):
                    ps = p256.tile([128, NL], F32, tag="p256", name=f"qp{li}{u}{mo}")
                    for k in range(2):
                        nc.tensor.matmul(ps[:],
                                         _r(wq_t[:, k, mo * 128:(mo + 1) * 128]),
                                         _r(xcur[:, u, k, :]),
                                         start=(k == 0), stop=(k == 1))
                    nc.vector.tensor_scalar(_r(qtile[:, mo, :]), ps[:],
                                            bq_c[:, mo:mo + 1], None, OP.add)
                qt[u] = qtile

            def kv_unit(u, src):
                ktile = a1.tile([128, 2, N], F32, tag=f"k{u}", name=f"k{li}{u}")
                for mo in range(2):
                    ps = p4.tile([128, 4, NL], F32, tag="p4", name=f"kp{li}{u}{mo}")
                    psv = ps[:].rearrange("p a b -> p (a b)").rearrange(
                        "p (n c) -> p n c", n=2)
                    for nn in range(2):
                        for k in range(2):
                            nc.tensor.matmul(psv[:, nn, :],
                                             wk_t[:, k, mo * 128:(mo + 1) * 128],
                                             src[:, k, nn * 512:(nn + 1) * 512],
                                             start=(k == 0), stop=(k == 1))
                    nc.vector.tensor_copy(
                        _r(ktile[:, mo, :]),
                        ps[:].rearrange("p a b -> p (a b)"))
                kt[u] = ktile
                for f in range(8):
                    ps = p256.tile([128, 256], F32, tag="p256", name=f"vp{li}{u}{f}")
                    for k in range(2):
                        nc.tensor.matmul(ps[:], src[:, k, f * 128:(f + 1) * 128],
                                         wv_t[:, k, :], start=(k == 0), stop=(k == 1))
                    tv = vt[u][f][:].rearrange("p (h c) -> p h c", h=4)
                    nc.vector.tensor_copy(_r(tv[:, :, 0:64]),
                                          ps[:].rearrange("p (h c) -> p h c", c=64))

            def attn_unit(u):
                msgt[u] = a2.tile([128, 2, NL], F32, tag=f"m{u}", name=f"m{li}{u}")
                for h in range(H):
                    kt_t = kt[u]
                    hc = h // 2
                    r0 = (h % 2) * 64
                    mg = pmsg.tile([65, NL], F32, tag="pmsg", name=f"mg{li}{u}{h}")
                    # two 4-wide score blocks; exp of block a overlaps block
                    # b's matmuls, msg matmuls run once the exp lands
                    exs = []
                    for blk in range(2):
                        sc = p4.tile([128, 4, NL], F32, tag="p4",
                                     name=f"sc{li}{u}{h}{blk}")
                        for j in range(4):
                            f = blk * 4 + j
                            nc.tensor.matmul(
                                sc[:, j, :],
                                _r(kt_t[r0:r0 + 64, hc, f * 128:(f + 1) * 128]),
                                _r(qt[u][r0:r0 + 64, hc, :]),
                                start=True, stop=True)
                        ex = ep.tile([128, 4, NL], F32, tag="ep",
                                     name=f"ex{li}{u}{h}{blk}")
                        nc.scalar.activation(_r(ex[:]), sc[:], AF.Exp)
                        exs.append(ex)
                    for blk in range(2):
                        for j in range(4):
                            f = blk * 4 + j
                            nc.tensor.matmul(mg[:],
                                             _r(vt[u][f][:, h * 65:(h + 1) * 65]),
                                             _r(exs[blk][:, j, :]),
                                             start=(f == 0), stop=(f == 7))
                    rec = a2.tile([1, NL], F32, tag="rec", name=f"rec{li}{u}{h}")
                    nc.vector.reciprocal(rec[:], mg[64:65, :])
                    rbc = a2.tile([64, NL], F32, tag="rbc", name=f"rbc{li}{u}{h}")
                    nc.gpsimd.partition_broadcast(rbc[:], rec[:])
                    nc.vector.tensor_tensor(_r(msgt[u][r0:r0 + 64, hc, :]),
                                            mg[0:64, :], rbc[:], OP.mult)

            def _stats_exchange(us):
                tg = "".join(str(u) for u in us)
                nu = len(us)
                bni = dp.tile([128, nu, 2, 4], F32, tag=f"bni{tg}",
                              name=f"bni{li}{tg}")
                bno = dp.tile([8, 128, nu, 2, 4], F32, tag=f"bno{tg}",
                              name=f"bno{li}{tg}")
                for j, u in enumerate(us):
                    nc.gpsimd.dma_start(out=bni[:, j], in_=stl_d[u][:])
                if use_coll:
                    nc.gpsimd.collective_compute("AllGather", OP.bypass,
                                                 replica_groups=RG_ALL,
                                                 ins=[bni[:].opt()],
                                                 outs=[bno[:].opt()])
                else:
                    nc.sync.dma_start(out=bno[0], in_=bni[:])
                sg8 = a2.tile([128, 8, nu, 2, 4], F32, tag=f"sg8{tg}",
                              name=f"sg8{li}{tg}")
                nc.gpsimd.dma_start(
                    out=sg8[:], in_=bno[:].rearrange("g p u s m -> p g u s m"))
                s4 = a2.tile([128, 4, nu, 2, 4], F32, tag=f"s4{tg}",
                             name=f"s4{li}{tg}")
                nc.gpsimd.tensor_add(s4[:], sg8[:, 0:4], sg8[:, 4:8])
                s2 = a2.tile([128, 2, nu, 2, 4], F32, tag=f"s2{tg}",
                             name=f"s2{li}{tg}")
                nc.gpsimd.tensor_add(s2[:], s4[:, 0:2], s4[:, 2:4])
                sg = a2.tile([128, nu, 2, 4], F32, tag=f"stg{tg}",
                             name=f"stg{li}{tg}")
                nc.gpsimd.tensor_add(sg[:], s2[:, 0], s2[:, 1])
                for j, u in enumerate(us):
                    stg[u] = sg[:, j]

            def mlp_unit(u):
                msgc = a2.tile([128, 2, NL], F32, tag=f"mc{u}", name=f"mc{li}{u}")
                for mo in range(2):
                    ps = p256.tile([128, NL], F32, tag="p256", name=f"cp{li}{u}{mo}")
                    for k in range(2):
                        nc.tensor.matmul(ps[:],
                                         _r(wm_t[:, k, mo * 128:(mo + 1) * 128]),
                                         _r(msgt[u][:, k, :]),
                                         start=(k == 0), stop=(k == 1))
                    nc.vector.tensor_scalar(_r(msgc[:, mo, :]), ps[:],
                                            bm_c[:, mo:mo + 1], None, OP.add)
                ych = [xcur[:, u, 0, :], xcur[:, u, 1, :],
                       msgc[:, 0, :], msgc[:, 1, :]]
                stl = a2.tile([128, 2, 4], F32, tag=f"stl{u}", name=f"stl{li}{u}")
                htile = a1.tile([128, 4, NL], F32, tag=f"h{u}", name=f"h{li}{u}")
                for mo in range(4):
                    ps = p256.tile([128, NL], F32, tag="p256", name=f"h1p{li}{u}{mo}")
                    for k in range(4):
                        nc.tensor.matmul(ps[:],
                                         _r(w1_t[:, k, mo * 128:(mo + 1) * 128]),
                                         _r(ych[k]), start=(k == 0), stop=(k == 3))
                    nc.scalar.activation(htile[:, mo, :], ps[:], AF.Identity,
                                         bias=b1_c[:, mo:mo + 1],
                                         accum_out=stl[:, 0, mo:mo + 1])
                    sqs = ep.tile([128, NL], F32, tag="sq", name=f"sq{li}{u}{mo}")
                    nc.scalar.activation(sqs[:], htile[:, mo, :], AF.Square,
                                         accum_out=stl[:, 1, mo:mo + 1])
                ht[u] = htile
                stl_d[u] = stl
                if not merge_stats:
                    _stats_exchange((u,))

            def make_cx(u):
                return dict(u=u, li=li, i=i, npar=npar, xn=xn, xn16=xn16,
                            stg=stg[u][:], ht=ht[u], resid=resid, w2=w2_t,
                            b2=b2_c, g1=g1_c, be1=be1_c,
                            last=(li == n_layers - 1))

            kv_unit(uA, slabs[sA])
            flush_pending()
            q_unit(uA)
            q_unit(uB)
            attn_unit(uA)
            mlp_unit(uA)
            kv_unit(uB, slabs[sB])
            def _tail_A():
                cxA = make_cx(uA)
                bn_conv2(cxA)
                if li < n_layers - 1:
                    gather(cxA)
            if merge_stats:
                attn_unit(uB)
                mlp_unit(uB)
                _stats_exchange((uA, uB))
                _tail_A()
            elif bnA_pos == "after_kvB":
                _tail_A()
                attn_unit(uB)
                mlp_unit(uB)
            elif bnA_pos == "after_attnB":
                attn_unit(uB)
                _tail_A()
                mlp_unit(uB)
            else:  # after_mlpB
                attn_unit(uB)
                mlp_unit(uB)
                _tail_A()
            state["pending"] = make_cx(uB)
            state["xprev"] = xn
            if not defer:
                flush_pending()

        flush_pending()
        _es.close()

    nc.finalize()
    return nc


def _host_prep(inputs):
    f = np.float32
    Wq, bq = np.asarray(inputs["Wq"], f), np.asarray(inputs["bq"], f)
    Wk = np.asarray(inputs["Wk"], f)
    Wv, bv = np.asarray(inputs["Wv"], f), np.asarray(inputs["bv"], f)
    Wm, bm = np.asarray(inputs["Wm"], f), np.asarray(inputs["bm"], f)
    W1, b1 = np.asarray(inputs["W1"], f), np.asarray(inputs["b1"], f)
    g1, be1 = np.asarray(inputs["g1"], f), np.asarray(inputs["be1"], f)
    W2, b2 = np.asarray(inputs["W2"], f), np.asarray(inputs["b2"], f)
    d0, d1 = np.asarray(inputs["desc0"], f), np.asarray(inputs["desc1"], f)

    SCALE = f(1.0 / np.sqrt(HD))

    def lhsT(w, kc=2):
        # w: [L, out, in] -> partition-major lhsT [L, 128, kc, out]
        t = w.transpose(0, 2, 1).reshape(L, kc, 128, w.shape[1])
        return np.ascontiguousarray(t.transpose(0, 2, 1, 3))

    wqt = lhsT(Wq[:, PERM, :] * SCALE)
    wkt = lhsT(Wk[:, PERM, :])
    wvt = lhsT(Wv[:, PERM, :])            # rhs [in-chunks, out_perm] — same form
    wmt = lhsT(Wm[:, :, PERM])
    wqm = np.ascontiguousarray(np.stack([wqt, wmt], axis=3))
    wkv = np.ascontiguousarray(
        np.stack([wkt, wvt], axis=3).astype(NP_BF16))
    w1t = lhsT(W1, kc=4)
    w2t = lhsT(W2, kc=4)

    bq_a = (bq[:, PERM] * SCALE).reshape(L, 2, 128).transpose(0, 2, 1)
    bm_eff = (np.einsum("loi,li->lo", Wm, bv) + bm).astype(f)
    bm_a = bm_eff.reshape(L, 2, 128).transpose(0, 2, 1)
    b1_a = b1.reshape(L, 4, 128).transpose(0, 2, 1)
    b2_a = b2.reshape(L, 2, 128).transpose(0, 2, 1)
    g1_a = g1.reshape(L, 4, 128).transpose(0, 2, 1)
    be1_a = be1.reshape(L, 4, 128).transpose(0, 2, 1)
    bia = np.concatenate([bq_a, bm_a, b1_a, b2_a, g1_a, g1_a, be1_a, be1_a], axis=2)
    bia = np.ascontiguousarray(bia.astype(f))
    assert bia.shape == (L, 128, 26)

    shared = dict(wqm=wqm, wkv=wkv, w1t=w1t, w2t=w2t, bia=bia)
    in_maps = []
    for c in range(8):
        b, q = c // 4, c % 4
        m = dict(shared)
        dsc = np.stack([d0[b].reshape(2, 128, N), d1[b].reshape(2, 128, N)], axis=2)
        m["dsc"] = np.ascontiguousarray(dsc.transpose(1, 0, 2, 3)).astype(NP_BF16)
        dlo = np.stack([d0[b][:, q * NL:(q + 1) * NL].reshape(2, 128, NL),
                        d1[b][:, q * NL:(q + 1) * NL].reshape(2, 128, NL)], axis=0)
        m["dlo"] = np.ascontiguousarray(dlo.transpose(2, 0, 1, 3))
        in_maps.append(m)
    return in_maps, d0, d1


class _Exec:
    """Compiled 8-core executable: jit cached across kernel() calls."""

    def __init__(self, n_layers=L):
        import jax
        from jax.experimental.shard_map import shard_map
        from jax.sharding import Mesh, PartitionSpec

        nc = _build_program(n_layers=n_layers)
        bass2jax.install_neuronx_cc_hook()
        partition_name = (nc.partition_id_tensor.name
                          if nc.partition_id_tensor else None)
        in_names, out_names, out_avals = [], [], []
        for alloc in nc.m.functions[0].allocations:
            if not isinstance(alloc, mybir.MemoryLocationSet):
                continue
            name = alloc.memorylocations[0].name
            if alloc.kind == "ExternalInput":
                if name != partition_name:
                    in_names.append(name)
            elif alloc.kind == "ExternalOutput":
                assert alloc.tensor_shape is not None and alloc.dtype is not None
                out_names.append(name)
                out_avals.append(jax.core.ShapedArray(
                    tuple(alloc.tensor_shape), mybir.dt.np(alloc.dtype)))
        assert nc.dbg_addr is None
        n_params = len(in_names)
        all_names = tuple(in_names) + tuple(out_names)
        if partition_name is not None:
            all_names = all_names + (partition_name,)
        out_avals_t = tuple(out_avals)
        out_names_t = tuple(out_names)

        def _body(*args):
            operands = list(args)
            if partition_name is not None:
                operands.append(bass2jax.partition_id_tensor())
            outs = bass2jax._bass_exec_p.bind(
                *operands,
                out_avals=out_avals_t,
                in_names=all_names,
                out_names=out_names_t,
                lowering_input_output_aliases=(),
                sim_require_finite=True,
                sim_require_nnan=True,
                nc=nc,
            )
            return tuple(outs)

        devices = jax.devices()[:8]
        assert len(devices) == 8
        self.mesh = Mesh(np.asarray(devices), ("core",))
        self.pspec = PartitionSpec("core")
        n_args = n_params + len(out_names)
        donate = tuple(range(n_params, n_args))
        self.sharded = jax.jit(
            shard_map(_body, mesh=self.mesh,
                      in_specs=(self.pspec,) * n_args,
                      out_specs=(self.pspec,) * len(out_names),
                      check_rep=False),
            donate_argnums=donate, keep_unused=True)
        self.nc = nc
        self.in_names = in_names
        self.out_names = out_names
        self.out_avals = out_avals
        self.n_params = n_params

    def concat_inputs(self, in_maps):
        return [np.concatenate([np.asarray(m[name]) for m in in_maps], axis=0)
                for name in self.in_names]

    def make_zeros_host(self):
        return [np.zeros((8 * a.shape[0], *a.shape[1:]), a.dtype)
                for a in self.out_avals]

    def run(self, concat_in, zeros):
        import jax
        outs = self.sharded(*concat_in, *zeros)
        jax.block_until_ready(outs)
        return outs

    def split_outputs(self, outs):
        res = []
        for c in range(8):
            m = {}
            for i, name in enumerate(self.out_names):
                a = np.asarray(outs[i])
                m[name] = a.reshape(8, *self.out_avals[i].shape)[c]
            res.append(m)
        return res


def _get_exec(n_layers=L):
    key = f"exec{n_layers}"
    if key not in _CACHE:
        _CACHE[key] = _Exec(n_layers=n_layers)
    return _CACHE[key]


def kernel(**inputs):
    ex = _get_exec()
    in_maps, d0, d1 = _host_prep(inputs)
    concat_in = ex.concat_inputs(in_maps)
    outs = ex.run(concat_in, ex.make_zeros_host())
    results = ex.split_outputs(outs)

    full = [np.zeros((B, D, N), np.float32) for _ in range(2 * L + 2)]
    full[2] = d0.copy(); full[3] = d1.copy()
    for c in range(8):
        b, q = c // 4, c % 4
        O = results[c]["out"]  # [L, 128, 2, 2, NL]
        for i in range(L):
            for u in range(2):
                j = u if i == 0 else (4 + u if i == 1 else 2 * i + 2 + u)
                full[j][b, :, q * NL:(q + 1) * NL] = \
                    O[i, :, u].transpose(1, 0, 2).reshape(D, NL)
    return tuple(full)


# revision 21
# speedup vs baseline: 1.2192x; 1.2192x over previous
"""AttentionalGNN Trainium2 kernel — 8-core SPMD.

Sharding: core c = (b, q) with b = c // 4 (batch), q = c % 4 (node quarter,
256 of 1024 nodes). Every core runs an identical program; per-core behavior
differs only through input data (its batch's desc tensors and its node
slice). Per layer, 3 collectives:
  - ONE merged BatchNorm-stats exchange for both streams: an 8-way
    AllGather of the per-core sum/sumsq accumulators plus a local sum-tree
    on the Pool engine (AllGather + local reduce has lower latency than
    AllReduce for tiny payloads, and merging both streams halves the
    fixed collective overhead),
  - TWO per-stream bf16 AllGathers of the layer output within each batch
    group of 4, staggered so the next layer's first attention unit starts
    as soon as its source slab lands (unit processing order follows
    _unit_order so the first-needed slab is always the first gathered).
k/v^T convs run in bf16 over the full node axis (replicated within a batch
group, from the bf16 gathered slabs); q/attention/MLP run as float32r
(full-rate fp32) on the local node quarter. Scores are computed in two
4-wide PSUM blocks per head with one Exp activation each, so the exp of
block a overlaps block b's matmuls. Softmax uses no max-subtraction
(|scores| <= ~64 for this model, exp stays in fp32 range); the per-node
softmax denominator comes from a ones-column folded into v^T, and the
division is applied during the message-PSUM evacuation. All DRAM layouts
are partition-major so every DMA is per-partition contiguous.

The compiled executable (jax.jit of a shard_map'd bass_exec call) is cached
in-process so repeated kernel() calls skip retracing and recompilation.
"""

import numpy as np
import ml_dtypes

import concourse.bass as bass
import concourse.tile as tile
from concourse import bacc, bass2jax, mybir

L, D, H, B, N = 18, 256, 4, 2, 1024
HD = D // H           # 64
NL = N // 4           # 256 local nodes per core
EPS = 1e-5
F32 = mybir.dt.float32
F32R = mybir.dt.float32r
BF16 = mybir.dt.bfloat16
I32 = mybir.dt.int32
AF = mybir.ActivationFunctionType
OP = mybir.AluOpType
NP_BF16 = ml_dtypes.bfloat16

# head-contiguous channel permutation: perm[h*64+hd] = hd*4+h
PERM = np.array([hd * H + h for h in range(H) for hd in range(HD)], np.int64)

_CACHE = {}


def _r(ap):
    return ap.bitcast(F32R)


def _unit_order(li):
    # process first the unit whose source slab arrives first (see AG stagger)
    return (0, 1) if li % 4 in (0, 3) else (1, 0)


def _build_program(n_layers=L, use_coll=True, num_devices=8,
                   bnA_pos="after_attnB", merge_stats=True, defer=False):
    nc = bacc.Bacc("TRN2", target_bir_lowering=False, debug=False,
                   num_devices=num_devices)

    dram = {}
    def din(name, shape, dt=F32):
        dram[name] = nc.dram_tensor(name, shape, dt, kind="ExternalInput")
    din("wqm", [L, 128, 2, 2, 256])        # (p, k, {q,m}, o) fp32
    din("wkv", [L, 128, 2, 2, 256], BF16)  # (p, k, {k,v}, o) bf16
    din("w1t", [L, 128, 4, 512])
    din("w2t", [L, 128, 4, 256])
    din("bia", [L, 128, 26])               # bq2 bm2 b1(4) b2(2) g1(8) be1(8)
    din("dsc", [128, 2, 2, N], BF16)       # (p, chunk, stream, n) full descs
    din("dlo", [128, 2, 2, NL])            # (p, stream, chunk, n) local slices
    out_d = nc.dram_tensor("out", [L, 128, 2, 2, NL], F32, kind="ExternalOutput")

    RG_ALL = [list(range(8))]
    RG_B = [[0, 1, 2, 3], [4, 5, 6, 7]]

    with tile.TileContext(nc) as tc:
        from contextlib import ExitStack
        _es = ExitStack()
        wp = _es.enter_context(tc.tile_pool(name="wp", bufs=2))
        a2 = _es.enter_context(tc.tile_pool(name="a2", bufs=2))
        a1 = _es.enter_context(tc.tile_pool(name="a1", bufs=1))
        ep = _es.enter_context(tc.tile_pool(name="ep", bufs=4))
        p4 = _es.enter_context(tc.tile_pool(name="p4", bufs=2, space="PSUM"))
        p256 = _es.enter_context(tc.tile_pool(name="p256", bufs=2, space="PSUM"))
        pmsg = _es.enter_context(tc.tile_pool(name="pmsg", bufs=2, space="PSUM"))
        dp = _es.enter_context(tc.tile_pool(name="dp", bufs=2, space="DRAM"))

        # ---- persistent tiles ----
        slab_t = [[a1.tile([128, 2, N], BF16, tag=f"sl{s}{pp}", name=f"sl{s}{pp}")
                   for pp in range(2)] for s in range(2)]
        for s in range(2):
            nc.sync.dma_start(out=slab_t[s][0][:],
                              in_=dram["dsc"].ap()[:, :, s, :])
        slabs = [slab_t[0][0], slab_t[1][0]]
        dl = a1.tile([128, 2, 2, NL], F32, tag="dl", name="dl")
        nc.sync.dma_start(out=_r(dl[:]), in_=_r(dram["dlo"].ap()))
        # vT tiles with persistent ones columns, double-buffered by parity
        vt_t = [[[a1.tile([128, 260], F32, tag=f"v{u}{f}{pp}", name=f"v{u}{f}{pp}")
                  for f in range(8)] for u in range(2)] for pp in range(2)]
        for pp in range(2):
            for u in range(2):
                for f in range(8):
                    tv = vt_t[pp][u][f][:].rearrange("p (h c) -> p h c", h=4)
                    nc.vector.memset(tv[:, :, 64:65], 1.0)

        # ---- deferred per-stream BN/conv2/gather (software pipelined across
        # layers: stream A's tail is emitted mid-layer behind stream B's
        # attention; stream B's tail is deferred into the next layer, hidden
        # behind its first k/v convs) ----
        state = {"pending": None, "xprev": None}

        def bn_conv2(cx):
            nc_ = nc
            u, li = cx["u"], cx["li"]
            stg_u, ht_u = cx["stg"], cx["ht"]
            xn, xn16 = cx["xn"], cx["xn16"]
            g1u = cx["g1"][:, u * 4:(u + 1) * 4]
            be1u = cx["be1"][:, u * 4:(u + 1) * 4]
            mean_t = a2.tile([128, 4], F32, tag=f"mean{u}", name=f"mean{li}{u}")
            var_t = a2.tile([128, 4], F32, tag=f"var{u}", name=f"var{li}{u}")
            sc_t = a2.tile([128, 4], F32, tag=f"scl{u}", name=f"scl{li}{u}")
            sh_t = a2.tile([128, 4], F32, tag=f"shf{u}", name=f"shf{li}{u}")
            nc_.vector.tensor_scalar(mean_t[:], stg_u[:, 0, :], 1.0 / 2048.0,
                                     None, OP.mult)
            nc_.vector.tensor_scalar(var_t[:], stg_u[:, 1, :], 1.0 / 2048.0,
                                     None, OP.mult)
            nc_.vector.tensor_tensor(sc_t[:], mean_t[:], mean_t[:], OP.mult)
            nc_.vector.tensor_tensor(var_t[:], var_t[:], sc_t[:], OP.subtract)
            nc_.vector.tensor_scalar(var_t[:], var_t[:], EPS, None, OP.add)
            # rsqrt via magic-constant seed + 2 Newton steps (DVE only,
            # avoids ACT Ln/Sqrt which would force activation-table swaps)
            y_t = a2.tile([128, 4], F32, tag=f"rsq{u}", name=f"rsq{li}{u}")
            t_t = a2.tile([128, 4], F32, tag=f"rst{u}", name=f"rst{li}{u}")
            nc_.vector.tensor_scalar(y_t[:].bitcast(I32), var_t[:].bitcast(I32),
                                     1, None, OP.logical_shift_right)
            nc_.vector.tensor_scalar(y_t[:].bitcast(I32), y_t[:].bitcast(I32),
                                     -1, 0x5f3759df, OP.mult, OP.add)
            for _newton in range(2):
                nc_.vector.tensor_tensor(t_t[:], y_t[:], y_t[:], OP.mult)
                nc_.vector.tensor_tensor(t_t[:], t_t[:], var_t[:], OP.mult)
                nc_.vector.tensor_scalar(t_t[:], t_t[:], -0.5, 1.5, OP.mult, OP.add)
                nc_.vector.tensor_tensor(y_t[:], y_t[:], t_t[:], OP.mult)
            nc_.vector.tensor_tensor(sc_t[:], y_t[:], g1u, OP.mult)
            nc_.vector.tensor_tensor(sh_t[:], mean_t[:], sc_t[:], OP.mult)
            nc_.vector.tensor_tensor(sh_t[:], be1u, sh_t[:], OP.subtract)
            hn = a1.tile([128, 4, NL], F32, tag=f"hn{u}", name=f"hn{li}{u}")
            for mo in range(4):
                nc_.scalar.activation(_r(hn[:, mo, :]), ht_u[:, mo, :], AF.Relu,
                                      bias=sh_t[:, mo:mo + 1],
                                      scale=sc_t[:, mo:mo + 1])
            for mo in range(2):
                ps = p256.tile([128, NL], F32, tag="p256", name=f"o2p{li}{u}{mo}")
                for k in range(4):
                    nc_.tensor.matmul(ps[:],
                                      _r(cx["w2"][:, k, mo * 128:(mo + 1) * 128]),
                                      _r(hn[:, k, :]), start=(k == 0), stop=(k == 3))
                nc_.vector.tensor_scalar(_r(xn[:, u, mo, :]), ps[:],
                                         cx["b2"][:, mo:mo + 1], None, OP.add)
            nc_.vector.tensor_tensor(_r(xn[:, u]), xn[:, u], cx["resid"][:, u],
                                     OP.add)
            nc_.vector.tensor_copy(xn16[:, u], xn[:, u])

        def gather(cx):
            u, li, npar = cx["u"], cx["li"], cx["npar"]
            agi = dp.tile([128, 2, NL], BF16, tag=f"agi{u}", name=f"agi{li}{u}")
            ago = dp.tile([4, 128, 2, NL], BF16, tag=f"ago{u}", name=f"ago{li}{u}")
            nc.gpsimd.dma_start(out=agi[:], in_=cx["xn16"][:, u])
            if use_coll:
                nc.gpsimd.collective_compute("AllGather", OP.bypass,
                                             replica_groups=RG_B,
                                             ins=[agi[:].opt()],
                                             outs=[ago[:].opt()])
            else:
                # timing-only fallback: satisfy the dependency with one
                # local DMA (values in groups 1..3 are garbage)
                nc.sync.dma_start(out=ago[0], in_=agi[:])
            t = slab_t[u][npar]
            for c in range(2):
                nc.sync.dma_start(
                    out=t[:, c, :].rearrange("p (q n) -> p q n", q=4),
                    in_=ago[:, :, c, :].rearrange("q p n -> p q n"))
            slabs[u] = t

        def flush_pending():
            cx = state["pending"]
            if cx is None:
                return
            state["pending"] = None
            bn_conv2(cx)
            if not cx["last"]:
                gather(cx)
            nc.gpsimd.dma_start(out=out_d.ap()[cx["i"]], in_=cx["xn"][:])

        for li in range(n_layers):
            i = li % L
            par = li % 2
            npar = (li + 1) % 2
            w4_t = wp.tile([128, 2, 2, 256], F32, tag="w4", name=f"w4_{li}")
            nc.sync.dma_start(out=_r(w4_t[:]), in_=_r(dram["wqm"].ap()[i]))
            wkv_t = wp.tile([128, 2, 2, 256], BF16, tag="wkv", name=f"wkv_{li}")
            nc.sync.dma_start(out=wkv_t[:], in_=dram["wkv"].ap()[i])
            w1_t = wp.tile([128, 4, 512], F32, tag="w1", name=f"w1_{li}")
            nc.sync.dma_start(out=_r(w1_t[:]), in_=_r(dram["w1t"].ap()[i]))
            w2_t = wp.tile([128, 4, 256], F32, tag="w2", name=f"w2_{li}")
            nc.sync.dma_start(out=_r(w2_t[:]), in_=_r(dram["w2t"].ap()[i]))
            bia_t = wp.tile([128, 26], F32, tag="bia", name=f"bia_{li}")
            nc.sync.dma_start(out=bia_t[:], in_=dram["bia"].ap()[i])
            wq_t = w4_t[:, :, 0, :]; wm_t = w4_t[:, :, 1, :]
            wk_t = wkv_t[:, :, 0, :]; wv_t = wkv_t[:, :, 1, :]
            bq_c = bia_t[:, 0:2]; bm_c = bia_t[:, 2:4]
            b1_c = bia_t[:, 4:8]; b2_c = bia_t[:, 8:10]
            g1_c = bia_t[:, 10:18]; be1_c = bia_t[:, 18:26]

            self_layer = (li % 2 == 0)
            order = _unit_order(li)
            uA, uB = order
            sA = uA if self_layer else 1 - uA
            sB = uB if self_layer else 1 - uB
            xcur = state["xprev"] if li > 0 else dl
            resid = dl if li <= 1 else state["xprev"]

            xn = a2.tile([128, 2, 2, NL], F32, tag="xn", name=f"xn{li}")
            xn16 = a2.tile([128, 2, 2, NL], BF16, tag="xn16", name=f"xn16{li}")
            qt = [None, None]
            kt = [None, None]
            vt = vt_t[par]
            msgt = [None, None]
            stg = [None, None]
            stl_d = [None, None]
            ht = [None, None]

            def q_unit(u):
                qtile = a2.tile([128, 2, NL], F32, tag=f"q{u}", name=f"q{li}{u}")
                for mo in range(2):
                    ps = p256.tile([128, NL], F32, tag="p256", name=f"qp{li}{u}{mo}")
                    for k in range(2):
                        nc.tensor.matmul(ps[:],
                                         _r(wq_t[:, k, mo * 128:(mo + 1) * 128]),
                                         _r(xcur[:, u, k, :]),
                                         start=(k == 0), stop=(k == 1))
                    nc.vector.tensor_scalar(_r(qtile[:, mo, :]), ps[:],
                                            bq_c[:, mo:mo + 1], None, OP.add)
                qt[u] = qtile

            def kv_unit(u, src):
                ktile = a1.tile([128, 2, N], F32, tag=f"k{u}", name=f"k{li}{u}")
                for mo in range(2):
                    ps = p4.tile([128, 4, NL], F32, tag="p4", name=f"kp{li}{u}{mo}")
                    psv = ps[:].rearrange("p a b -> p (a b)").rearrange(
                        "p (n c) -> p n c", n=2)
                    for nn in range(2):
                        for k in range(2):
                            nc.tensor.matmul(psv[:, nn, :],
                                             wk_t[:, k, mo * 128:(mo + 1) * 128],
                                             src[:, k, nn * 512:(nn + 1) * 512],
                                             start=(k == 0), stop=(k == 1))
                    nc.vector.tensor_copy(
                        _r(ktile[:, mo, :]),
                        ps[:].rearrange("p a b -> p (a b)"))
                kt[u] = ktile
                for f in range(8):
                    ps = p256.tile([128, 256], F32, tag="p256", name=f"vp{li}{u}{f}")
                    for k in range(2):
                        nc.tensor.matmul(ps[:], src[:, k, f * 128:(f + 1) * 128],
                                         wv_t[:, k, :], start=(k == 0), stop=(k == 1))
                    tv = vt[u][f][:].rearrange("p (h c) -> p h c", h=4)
                    nc.vector.tensor_copy(_r(tv[:, :, 0:64]),
                                          ps[:].rearrange("p (h c) -> p h c", c=64))

            def attn_unit(u):
                msgt[u] = a2.tile([128, 2, NL], F32, tag=f"m{u}", name=f"m{li}{u}")
                for h in range(H):
                    kt_t = kt[u]
                    hc = h // 2
                    r0 = (h % 2) * 64
                    mg = pmsg.tile([65, NL], F32, tag="pmsg", name=f"mg{li}{u}{h}")
                    # two 4-wide score blocks; exp of block a overlaps block
                    # b's matmuls, msg matmuls run once the exp lands
                    exs = []
                    for blk in range(2):
                        sc = p4.tile([128, 4, NL], F32, tag="p4",
                                     name=f"sc{li}{u}{h}{blk}")
                        for j in range(4):
                            f = blk * 4 + j
                            nc.tensor.matmul(
                                sc[:, j, :],
                                _r(kt_t[r0:r0 + 64, hc, f * 128:(f + 1) * 128]),
                                _r(qt[u][r0:r0 + 64, hc, :]),
                                start=True, stop=True)
                        ex = ep.tile([128, 4, NL], F32, tag="ep",
                                     name=f"ex{li}{u}{h}{blk}")
                        nc.scalar.activation(_r(ex[:]), sc[:], AF.Exp)
                        exs.append(ex)
                    for blk in range(2):
                        for j in range(4):
                            f = blk * 4 + j
                            nc.tensor.matmul(mg[:],
                                             _r(vt[u][f][:, h * 65:(h + 1) * 65]),
                                             _r(exs[blk][:, j, :]),
                                             start=(f == 0), stop=(f == 7))
                    rec = a2.tile([1, NL], F32, tag="rec", name=f"rec{li}{u}{h}")
                    nc.vector.reciprocal(rec[:], mg[64:65, :])
                    rbc = a2.tile([64, NL], F32, tag="rbc", name=f"rbc{li}{u}{h}")
                    nc.gpsimd.partition_broadcast(rbc[:], rec[:])
                    nc.vector.tensor_tensor(_r(msgt[u][r0:r0 + 64, hc, :]),
                                            mg[0:64, :], rbc[:], OP.mult)

            def _stats_exchange(us):
                tg = "".join(str(u) for u in us)
                nu = len(us)
                bni = dp.tile([128, nu, 2, 4], F32, tag=f"bni{tg}",
                              name=f"bni{li}{tg}")
                bno = dp.tile([8, 128, nu, 2, 4], F32, tag=f"bno{tg}",
                              name=f"bno{li}{tg}")
                for j, u in enumerate(us):
                    nc.gpsimd.dma_start(out=bni[:, j], in_=stl_d[u][:])
                if use_coll:
                    nc.gpsimd.collective_compute("AllGather", OP.bypass,
                                                 replica_groups=RG_ALL,
                                                 ins=[bni[:].opt()],
                                                 outs=[bno[:].opt()])
                else:
                    nc.sync.dma_start(out=bno[0], in_=bni[:])
                sg8 = a2.tile([128, 8, nu, 2, 4], F32, tag=f"sg8{tg}",
                              name=f"sg8{li}{tg}")
                nc.gpsimd.dma_start(
                    out=sg8[:], in_=bno[:].rearrange("g p u s m -> p g u s m"))
                s4 = a2.tile([128, 4, nu, 2, 4], F32, tag=f"s4{tg}",
                             name=f"s4{li}{tg}")
                nc.gpsimd.tensor_add(s4[:], sg8[:, 0:4], sg8[:, 4:8])
                s2 = a2.tile([128, 2, nu, 2, 4], F32, tag=f"s2{tg}",
                             name=f"s2{li}{tg}")
                nc.gpsimd.tensor_add(s2[:], s4[:, 0:2], s4[:, 2:4])
                sg = a2.tile([128, nu, 2, 4], F32, tag=f"stg{tg}",
                             name=f"stg{li}{tg}")
                nc.gpsimd.tensor_add(sg[:], s2[:, 0], s2[:, 1])
                for j, u in enumerate(us):
                    stg[u] = sg[:, j]

            def mlp_unit(u):
                msgc = a2.tile([128, 2, NL], F32, tag=f"mc{u}", name=f"mc{li}{u}")
                for mo in range(2):
                    ps = p256.tile([128, NL], F32, tag="p256", name=f"cp{li}{u}{mo}")
                    for k in range(2):
                        nc.tensor.matmul(ps[:],
                                         _r(wm_t[:, k, mo * 128:(mo + 1) * 128]),
                                         _r(msgt[u][:, k, :]),
                                         start=(k == 0), stop=(k == 1))
                    nc.vector.tensor_scalar(_r(msgc[:, mo, :]), ps[:],
                                            bm_c[:, mo:mo + 1], None, OP.add)
                ych = [xcur[:, u, 0, :], xcur[:, u, 1, :],
                       msgc[:, 0, :], msgc[:, 1, :]]
                stl = a2.tile([128, 2, 4], F32, tag=f"stl{u}", name=f"stl{li}{u}")
                htile = a1.tile([128, 4, NL], F32, tag=f"h{u}", name=f"h{li}{u}")
                for mo in range(4):
                    ps = p256.tile([128, NL], F32, tag="p256", name=f"h1p{li}{u}{mo}")
                    for k in range(4):
                        nc.tensor.matmul(ps[:],
                                         _r(w1_t[:, k, mo * 128:(mo + 1) * 128]),
                                         _r(ych[k]), start=(k == 0), stop=(k == 3))
                    nc.scalar.activation(htile[:, mo, :], ps[:], AF.Identity,
                                         bias=b1_c[:, mo:mo + 1],
                                         accum_out=stl[:, 0, mo:mo + 1])
                    sqs = ep.tile([128, NL], F32, tag="sq", name=f"sq{li}{u}{mo}")
                    nc.scalar.activation(sqs[:], htile[:, mo, :], AF.Square,
                                         accum_out=stl[:, 1, mo:mo + 1])
                ht[u] = htile
                stl_d[u] = stl
                if not merge_stats:
                    _stats_exchange((u,))

            def make_cx(u):
                return dict(u=u, li=li, i=i, npar=npar, xn=xn, xn16=xn16,
                            stg=stg[u][:], ht=ht[u], resid=resid, w2=w2_t,
                            b2=b2_c, g1=g1_c, be1=be1_c,
                            last=(li == n_layers - 1))

            kv_unit(uA, slabs[sA])
            flush_pending()
            q_unit(uA)
            q_unit(uB)
            attn_unit(uA)
            mlp_unit(uA)
            kv_unit(uB, slabs[sB])
            def _tail_A():
                cxA = make_cx(uA)
                bn_conv2(cxA)
                if li < n_layers - 1:
                    gather(cxA)
            if merge_stats:
                attn_unit(uB)
                mlp_unit(uB)
                _stats_exchange((uA, uB))
                _tail_A()
            elif bnA_pos == "after_kvB":
                _tail_A()
                attn_unit(uB)
                mlp_unit(uB)
            elif bnA_pos == "after_attnB":
                attn_unit(uB)
                _tail_A()
                mlp_unit(uB)
            else:  # after_mlpB
                attn_unit(uB)
                mlp_unit(uB)
                _tail_A()
            state["pending"] = make_cx(uB)
            state["xprev"] = xn
            if not defer:
                flush_pending()

        flush_pending()
        _es.close()

    nc.finalize()
    return nc


def _host_prep(inputs):
    f = np.float32
    Wq, bq = np.asarray(inputs["Wq"], f), np.asarray(inputs["bq"], f)
    Wk = np.asarray(inputs["Wk"], f)
    Wv, bv = np.asarray(inputs["Wv"], f), np.asarray(inputs["bv"], f)
    Wm, bm = np.asarray(inputs["Wm"], f), np.asarray(inputs["bm"], f)
    W1, b1 = np.asarray(inputs["W1"], f), np.asarray(inputs["b1"], f)
    g1, be1 = np.asarray(inputs["g1"], f), np.asarray(inputs["be1"], f)
    W2, b2 = np.asarray(inputs["W2"], f), np.asarray(inputs["b2"], f)
    d0, d1 = np.asarray(inputs["desc0"], f), np.asarray(inputs["desc1"], f)

    SCALE = f(1.0 / np.sqrt(HD))

    def lhsT(w, kc=2):
        # w: [L, out, in] -> partition-major lhsT [L, 128, kc, out]
        t = w.transpose(0, 2, 1).reshape(L, kc, 128, w.shape[1])
        return np.ascontiguousarray(t.transpose(0, 2, 1, 3))

    wqt = lhsT(Wq[:, PERM, :] * SCALE)
    wkt = lhsT(Wk[:, PERM, :])
    wvt = lhsT(Wv[:, PERM, :])            # rhs [in-chunks, out_perm] — same form
    wmt = lhsT(Wm[:, :, PERM])
    wqm = np.ascontiguousarray(np.stack([wqt, wmt], axis=3))
    wkv = np.ascontiguousarray(
        np.stack([wkt, wvt], axis=3).astype(NP_BF16))
    w1t = lhsT(W1, kc=4)
    w2t = lhsT(W2, kc=4)

    bq_a = (bq[:, PERM] * SCALE).reshape(L, 2, 128).transpose(0, 2, 1)
    bm_eff = (np.einsum("loi,li->lo", Wm, bv) + bm).astype(f)
    bm_a = bm_eff.reshape(L, 2, 128).transpose(0, 2, 1)
    b1_a = b1.reshape(L, 4, 128).transpose(0, 2, 1)
    b2_a = b2.reshape(L, 2, 128).transpose(0, 2, 1)
    g1_a = g1.reshape(L, 4, 128).transpose(0, 2, 1)
    be1_a = be1.reshape(L, 4, 128).transpose(0, 2, 1)
    bia = np.concatenate([bq_a, bm_a, b1_a, b2_a, g1_a, g1_a, be1_a, be1_a], axis=2)
    bia = np.ascontiguousarray(bia.astype(f))
    assert bia.shape == (L, 128, 26)

    shared = dict(wqm=wqm, wkv=wkv, w1t=w1t, w2t=w2t, bia=bia)
    in_maps = []
    for c in range(8):
        b, q = c // 4, c % 4
        m = dict(shared)
        dsc = np.stack([d0[b].reshape(2, 128, N), d1[b].reshape(2, 128, N)], axis=2)
        m["dsc"] = np.ascontiguousarray(dsc.transpose(1, 0, 2, 3)).astype(NP_BF16)
        dlo = np.stack([d0[b][:, q * NL:(q + 1) * NL].reshape(2, 128, NL),
                        d1[b][:, q * NL:(q + 1) * NL].reshape(2, 128, NL)], axis=0)
        m["dlo"] = np.ascontiguousarray(dlo.transpose(2, 0, 1, 3))
        in_maps.append(m)
    return in_maps, d0, d1


class _Exec:
    """Compiled 8-core executable: jit cached across kernel() calls."""

    def __init__(self, n_layers=L):
        import jax
        from jax.experimental.shard_map import shard_map
        from jax.sharding import Mesh, PartitionSpec

        nc = _build_program(n_layers=n_layers)
        bass2jax.install_neuronx_cc_hook()
        partition_name = (nc.partition_id_tensor.name
                          if nc.partition_id_tensor else None)
        in_names, out_names, out_avals = [], [], []
        for alloc in nc.m.functions[0].allocations:
            if not isinstance(alloc, mybir.MemoryLocationSet):
                continue
            name = alloc.memorylocations[0].name
            if alloc.kind == "ExternalInput":
                if name != partition_name:
                    in_names.append(name)
            elif alloc.kind == "ExternalOutput":
                assert alloc.tensor_shape is not None and alloc.dtype is not None
                out_names.append(name)
                out_avals.append(jax.core.ShapedArray(
                    tuple(alloc.tensor_shape), mybir.dt.np(alloc.dtype)))
        assert nc.dbg_addr is None
        n_params = len(in_names)
        all_names = tuple(in_names) + tuple(out_names)
        if partition_name is not None:
            all_names = all_names + (partition_name,)
        out_avals_t = tuple(out_avals)
        out_names_t = tuple(out_names)

        def _body(*args):
            operands = list(args)
            if partition_name is not None:
                operands.append(bass2jax.partition_id_tensor())
            outs = bass2jax._bass_exec_p.bind(
                *operands,
                out_avals=out_avals_t,
                in_names=all_names,
                out_names=out_names_t,
                lowering_input_output_aliases=(),
                sim_require_finite=True,
                sim_require_nnan=True,
                nc=nc,
            )
            return tuple(outs)

        devices = jax.devices()[:8]
        assert len(devices) == 8
        self.mesh = Mesh(np.asarray(devices), ("core",))
        self.pspec = PartitionSpec("core")
        n_args = n_params + len(out_names)
        donate = tuple(range(n_params, n_args))
        self.sharded = jax.jit(
            shard_map(_body, mesh=self.mesh,
                      in_specs=(self.pspec,) * n_args,
                      out_specs=(self.pspec,) * len(out_names),
                      check_rep=False),
            donate_argnums=donate, keep_unused=True)
        self.nc = nc
        self.in_names = in_names
        self.out_names = out_names
        self.out_avals = out_avals
        self.n_params = n_params

    def concat_inputs(self, in_maps):
        return [np.concatenate([np.asarray(m[name]) for m in in_maps], axis=0)
                for name in self.in_names]

    def make_zeros_host(self):
        return [np.zeros((8 * a.shape[0], *a.shape[1:]), a.dtype)
                for a in self.out_avals]

    def run(self, concat_in, zeros):
        import jax
        outs = self.sharded(*concat_in, *zeros)
        jax.block_until_ready(outs)
        return outs

    def split_outputs(self, outs):
        res = []
        for c in range(8):
            m = {}
            for i, name in enumerate(self.out_names):
                a = np.asarray(outs[i])
                m[name] = a.reshape(8, *self.out_avals[i].shape)[c]
            res.append(m)
        return res


def _get_exec(n_layers=L):
    key = f"exec{n_layers}"
    if key not in _CACHE:
        _CACHE[key] = _Exec(n_layers=n_layers)
    return _CACHE[key]


def kernel(**inputs):
    ex = _get_exec()
    in_maps, d0, d1 = _host_prep(inputs)
    concat_in = ex.concat_inputs(in_maps)
    outs = ex.run(concat_in, ex.make_zeros_host())
    results = ex.split_outputs(outs)

    full = [np.zeros((B, D, N), np.float32) for _ in range(2 * L + 2)]
    full[2] = d0.copy(); full[3] = d1.copy()
    for c in range(8):
        b, q = c // 4, c % 4
        O = results[c]["out"]  # [L, 128, 2, 2, NL]
        for i in range(L):
            for u in range(2):
                j = u if i == 0 else (4 + u if i == 1 else 2 * i + 2 + u)
                full[j][b, :, q * NL:(q + 1) * NL] = \
                    O[i, :, u].transpose(1, 0, 2).reshape(D, NL)
    return tuple(full)


# revision 23
# speedup vs baseline: 1.2346x; 1.0126x over previous
"""AttentionalGNN Trainium2 kernel — 8-core SPMD.

Sharding: core c = (b, q) with b = c // 4 (batch), q = c % 4 (node quarter,
256 of 1024 nodes). Every core runs an identical program; per-core behavior
differs only through input data (its batch's desc tensors and its node
slice). Per layer, 3 collectives:
  - ONE merged BatchNorm-stats exchange for both streams: an 8-way
    AllGather of the per-core sum/sumsq accumulators plus a local sum-tree
    on the Pool engine (AllGather + local reduce has lower latency than
    AllReduce for tiny payloads, and merging both streams halves the
    fixed collective overhead),
  - TWO per-stream bf16 AllGathers of the layer output within each batch
    group of 4, staggered so the next layer's first attention unit starts
    as soon as its source slab lands (unit processing order follows
    _unit_order so the first-needed slab is always the first gathered).
k/v^T convs run in bf16 over the full node axis (replicated within a batch
group, from the bf16 gathered slabs); q/attention/MLP run as float32r
(full-rate fp32) on the local node quarter. Scores are computed in two
4-wide PSUM blocks per head with one Exp activation each, so the exp of
block a overlaps block b's matmuls. Softmax uses no max-subtraction
(|scores| <= ~64 for this model, exp stays in fp32 range); the per-node
softmax denominator comes from a ones-column folded into v^T, and the
division is applied during the message-PSUM evacuation. All DRAM layouts
are partition-major so every DMA is per-partition contiguous.

The compiled executable (jax.jit of a shard_map'd bass_exec call) is cached
in-process so repeated kernel() calls skip retracing and recompilation.
"""

import numpy as np
import ml_dtypes

import concourse.bass as bass
import concourse.tile as tile
from concourse import bacc, bass2jax, mybir

L, D, H, B, N = 18, 256, 4, 2, 1024
HD = D // H           # 64
NL = N // 4           # 256 local nodes per core
EPS = 1e-5
F32 = mybir.dt.float32
F32R = mybir.dt.float32r
BF16 = mybir.dt.bfloat16
I32 = mybir.dt.int32
AF = mybir.ActivationFunctionType
OP = mybir.AluOpType
NP_BF16 = ml_dtypes.bfloat16

# head-contiguous channel permutation: perm[h*64+hd] = hd*4+h
PERM = np.array([hd * H + h for h in range(H) for hd in range(HD)], np.int64)

_CACHE = {}


def _r(ap):
    return ap.bitcast(F32R)


def _unit_order(li):
    # process first the unit whose source slab arrives first (see AG stagger)
    return (0, 1) if li % 4 in (0, 3) else (1, 0)


def _build_program(n_layers=L, use_coll=True, num_devices=8,
                   bnA_pos="after_attnB", merge_stats=True, defer=False):
    nc = bacc.Bacc("TRN2", target_bir_lowering=False, debug=False,
                   num_devices=num_devices)

    dram = {}
    def din(name, shape, dt=F32):
        dram[name] = nc.dram_tensor(name, shape, dt, kind="ExternalInput")
    din("wqm", [L, 128, 2, 2, 256])        # (p, k, {q,m}, o) fp32
    din("wkv", [L, 128, 2, 2, 256], BF16)  # (p, k, {k,v}, o) bf16
    din("w1t", [L, 128, 4, 512])
    din("w2t", [L, 128, 4, 256])
    din("bia", [L, 128, 26])               # bq2 bm2 b1(4) b2(2) g1(8) be1(8)
    din("dsc", [128, 2, 2, N], BF16)       # (p, chunk, stream, n) full descs
    din("dlo", [128, 2, 2, NL])            # (p, stream, chunk, n) local slices
    out_d = nc.dram_tensor("out", [L, 128, 2, 2, NL], F32, kind="ExternalOutput")

    RG_ALL = [list(range(8))]
    RG_B = [[0, 1, 2, 3], [4, 5, 6, 7]]

    with tile.TileContext(nc) as tc:
        from contextlib import ExitStack
        _es = ExitStack()
        wp = _es.enter_context(tc.tile_pool(name="wp", bufs=2))
        a2 = _es.enter_context(tc.tile_pool(name="a2", bufs=2))
        a1 = _es.enter_context(tc.tile_pool(name="a1", bufs=1))
        ep = _es.enter_context(tc.tile_pool(name="ep", bufs=4))
        p4 = _es.enter_context(tc.tile_pool(name="p4", bufs=2, space="PSUM"))
        p256 = _es.enter_context(tc.tile_pool(name="p256", bufs=2, space="PSUM"))
        pmsg = _es.enter_context(tc.tile_pool(name="pmsg", bufs=2, space="PSUM"))
        dp = _es.enter_context(tc.tile_pool(name="dp", bufs=2, space="DRAM"))

        # ---- persistent tiles ----
        slab_t = [[a1.tile([128, 2, N], BF16, tag=f"sl{s}{pp}", name=f"sl{s}{pp}")
                   for pp in range(2)] for s in range(2)]
        for s in range(2):
            nc.sync.dma_start(out=slab_t[s][0][:],
                              in_=dram["dsc"].ap()[:, :, s, :])
        slabs = [slab_t[0][0], slab_t[1][0]]
        dl = a1.tile([128, 2, 2, NL], F32, tag="dl", name="dl")
        nc.sync.dma_start(out=_r(dl[:]), in_=_r(dram["dlo"].ap()))
        # vT tiles with persistent ones columns, double-buffered by parity
        vt_t = [[[a1.tile([128, 260], F32, tag=f"v{u}{f}{pp}", name=f"v{u}{f}{pp}")
                  for f in range(8)] for u in range(2)] for pp in range(2)]
        for pp in range(2):
            for u in range(2):
                for f in range(8):
                    tv = vt_t[pp][u][f][:].rearrange("p (h c) -> p h c", h=4)
                    nc.vector.memset(tv[:, :, 64:65], 1.0)

        # ---- deferred per-stream BN/conv2/gather (software pipelined across
        # layers: stream A's tail is emitted mid-layer behind stream B's
        # attention; stream B's tail is deferred into the next layer, hidden
        # behind its first k/v convs) ----
        state = {"pending": None, "xprev": None}

        def bn_scalars(sg, us, li, g1_c, be1_c):
            # batched BN scale/shift for all exchanged streams at once:
            # sg is [128, nu, 2, 4] (unit, {sum,sumsq}, mo)
            nu = len(us)
            tg = "".join(str(u) for u in us)
            assert us == tuple(range(nu)) or nu == 1
            if nu == 1:
                u = us[0]
                g1v = g1_c[:, u * 4:(u + 1) * 4].rearrange("p (u m) -> p u m", u=1)
                bev = be1_c[:, u * 4:(u + 1) * 4].rearrange("p (u m) -> p u m", u=1)
            else:
                g1v = g1_c.rearrange("p (u m) -> p u m", u=2)
                bev = be1_c.rearrange("p (u m) -> p u m", u=2)
            mean_t = a2.tile([128, nu, 4], F32, tag=f"mean{tg}", name=f"mean{li}{tg}")
            var_t = a2.tile([128, nu, 4], F32, tag=f"var{tg}", name=f"var{li}{tg}")
            sc_t = a2.tile([128, nu, 4], F32, tag=f"scl{tg}", name=f"scl{li}{tg}")
            sh_t = a2.tile([128, nu, 4], F32, tag=f"shf{tg}", name=f"shf{li}{tg}")
            nc.vector.tensor_scalar(mean_t[:], sg[:, :, 0, :], 1.0 / 2048.0,
                                    None, OP.mult)
            nc.vector.tensor_scalar(var_t[:], sg[:, :, 1, :], 1.0 / 2048.0,
                                    None, OP.mult)
            nc.vector.tensor_tensor(sc_t[:], mean_t[:], mean_t[:], OP.mult)
            nc.vector.tensor_tensor(var_t[:], var_t[:], sc_t[:], OP.subtract)
            nc.vector.tensor_scalar(var_t[:], var_t[:], EPS, None, OP.add)
            # rsqrt via magic-constant seed + 2 Newton steps (DVE only,
            # avoids ACT Ln/Sqrt which would force activation-table swaps)
            y_t = a2.tile([128, nu, 4], F32, tag=f"rsq{tg}", name=f"rsq{li}{tg}")
            t_t = a2.tile([128, nu, 4], F32, tag=f"rst{tg}", name=f"rst{li}{tg}")
            nc.vector.tensor_scalar(y_t[:].bitcast(I32), var_t[:].bitcast(I32),
                                    1, None, OP.logical_shift_right)
            nc.vector.tensor_scalar(y_t[:].bitcast(I32), y_t[:].bitcast(I32),
                                    -1, 0x5f3759df, OP.mult, OP.add)
            for _newton in range(2):
                nc.vector.tensor_tensor(t_t[:], y_t[:], y_t[:], OP.mult)
                nc.vector.tensor_tensor(t_t[:], t_t[:], var_t[:], OP.mult)
                nc.vector.tensor_scalar(t_t[:], t_t[:], -0.5, 1.5, OP.mult, OP.add)
                nc.vector.tensor_tensor(y_t[:], y_t[:], t_t[:], OP.mult)
            nc.vector.tensor_tensor(sc_t[:], y_t[:], g1v, OP.mult)
            nc.vector.tensor_tensor(sh_t[:], mean_t[:], sc_t[:], OP.mult)
            nc.vector.tensor_tensor(sh_t[:], bev, sh_t[:], OP.subtract)
            return sc_t, sh_t

        def bn_conv2(cx):
            nc_ = nc
            u, li = cx["u"], cx["li"]
            ht_u = cx["ht"]
            xn, xn16 = cx["xn"], cx["xn16"]
            sc_t = cx["sc"]
            sh_t = cx["sh"]
            hn = a1.tile([128, 4, NL], F32, tag=f"hn{u}", name=f"hn{li}{u}")
            for mo in range(4):
                nc_.scalar.activation(_r(hn[:, mo, :]), ht_u[:, mo, :], AF.Relu,
                                      bias=sh_t[:, mo:mo + 1],
                                      scale=sc_t[:, mo:mo + 1])

            for mo in range(2):
                ps = p256.tile([128, NL], F32, tag="p256", name=f"o2p{li}{u}{mo}")
                for k in range(4):
                    nc_.tensor.matmul(ps[:],
                                      _r(cx["w2"][:, k, mo * 128:(mo + 1) * 128]),
                                      _r(hn[:, k, :]), start=(k == 0), stop=(k == 3))
                nc_.vector.tensor_scalar(_r(xn[:, u, mo, :]), ps[:],
                                         cx["b2"][:, mo:mo + 1], None, OP.add)
            nc_.vector.tensor_tensor(_r(xn[:, u]), xn[:, u], cx["resid"][:, u],
                                     OP.add)
            nc_.vector.tensor_copy(xn16[:, u], xn[:, u])

        def gather(cx):
            u, li, npar = cx["u"], cx["li"], cx["npar"]
            agi = dp.tile([128, 2, NL], BF16, tag=f"agi{u}", name=f"agi{li}{u}")
            ago = dp.tile([4, 128, 2, NL], BF16, tag=f"ago{u}", name=f"ago{li}{u}")
            nc.gpsimd.dma_start(out=agi[:], in_=cx["xn16"][:, u])
            if use_coll:
                nc.gpsimd.collective_compute("AllGather", OP.bypass,
                                             replica_groups=RG_B,
                                             ins=[agi[:].opt()],
                                             outs=[ago[:].opt()])
            else:
                # timing-only fallback: satisfy the dependency with one
                # local DMA (values in groups 1..3 are garbage)
                nc.sync.dma_start(out=ago[0], in_=agi[:])
            t = slab_t[u][npar]
            for c in range(2):
                nc.sync.dma_start(
                    out=t[:, c, :].rearrange("p (q n) -> p q n", q=4),
                    in_=ago[:, :, c, :].rearrange("q p n -> p q n"))
            slabs[u] = t

        def flush_pending():
            cx = state["pending"]
            if cx is None:
                return
            state["pending"] = None
            bn_conv2(cx)
            if not cx["last"]:
                gather(cx)
            nc.gpsimd.dma_start(out=out_d.ap()[cx["i"]], in_=cx["xn"][:])

        for li in range(n_layers):
            i = li % L
            par = li % 2
            npar = (li + 1) % 2
            w4_t = wp.tile([128, 2, 2, 256], F32, tag="w4", name=f"w4_{li}")
            nc.sync.dma_start(out=_r(w4_t[:]), in_=_r(dram["wqm"].ap()[i]))
            wkv_t = wp.tile([128, 2, 2, 256], BF16, tag="wkv", name=f"wkv_{li}")
            nc.sync.dma_start(out=wkv_t[:], in_=dram["wkv"].ap()[i])
            w1_t = wp.tile([128, 4, 512], F32, tag="w1", name=f"w1_{li}")
            nc.sync.dma_start(out=_r(w1_t[:]), in_=_r(dram["w1t"].ap()[i]))
            w2_t = wp.tile([128, 4, 256], F32, tag="w2", name=f"w2_{li}")
            nc.sync.dma_start(out=_r(w2_t[:]), in_=_r(dram["w2t"].ap()[i]))
            bia_t = wp.tile([128, 26], F32, tag="bia", name=f"bia_{li}")
            nc.sync.dma_start(out=bia_t[:], in_=dram["bia"].ap()[i])
            wq_t = w4_t[:, :, 0, :]; wm_t = w4_t[:, :, 1, :]
            wk_t = wkv_t[:, :, 0, :]; wv_t = wkv_t[:, :, 1, :]
            bq_c = bia_t[:, 0:2]; bm_c = bia_t[:, 2:4]
            b1_c = bia_t[:, 4:8]; b2_c = bia_t[:, 8:10]
            g1_c = bia_t[:, 10:18]; be1_c = bia_t[:, 18:26]

            self_layer = (li % 2 == 0)
            order = _unit_order(li)
            uA, uB = order
            sA = uA if self_layer else 1 - uA
            sB = uB if self_layer else 1 - uB
            xcur = state["xprev"] if li > 0 else dl
            resid = dl if li <= 1 else state["xprev"]

            xn = a2.tile([128, 2, 2, NL], F32, tag="xn", name=f"xn{li}")
            xn16 = a2.tile([128, 2, 2, NL], BF16, tag="xn16", name=f"xn16{li}")
            qt = [None, None]
            kt = [None, None]
            vt = vt_t[par]
            msgt = [None, None]
            stg = [None, None]
            scsh = [None, None]
            stl_d = [None, None]
            ht = [None, None]

            def q_unit(u):
                qtile = a2.tile([128, 2, NL], F32, tag=f"q{u}", name=f"q{li}{u}")
                for mo in range(2):
                    ps = p256.tile([128, NL], F32, tag="p256", name=f"qp{li}{u}{mo}")
                    for k in range(2):
                        nc.tensor.matmul(ps[:],
                                         _r(wq_t[:, k, mo * 128:(mo + 1) * 128]),
                                         _r(xcur[:, u, k, :]),
                                         start=(k == 0), stop=(k == 1))
                    nc.vector.tensor_scalar(_r(qtile[:, mo, :]), ps[:],
                                            bq_c[:, mo:mo + 1], None, OP.add)
                qt[u] = qtile

            def kv_unit(u, src):
                ktile = a1.tile([128, 2, N], F32, tag=f"k{u}", name=f"k{li}{u}")
                for mo in range(2):
                    ps = p4.tile([128, 4, NL], F32, tag="p4", name=f"kp{li}{u}{mo}")
                    psv = ps[:].rearrange("p a b -> p (a b)").rearrange(
                        "p (n c) -> p n c", n=2)
                    for nn in range(2):
                        for k in range(2):
                            nc.tensor.matmul(psv[:, nn, :],
                                             wk_t[:, k, mo * 128:(mo + 1) * 128],
                                             src[:, k, nn * 512:(nn + 1) * 512],
                                             start=(k == 0), stop=(k == 1))
                    nc.vector.tensor_copy(
                        _r(ktile[:, mo, :]),
                        ps[:].rearrange("p a b -> p (a b)"))
                kt[u] = ktile
                for f in range(8):
                    ps = p256.tile([128, 256], F32, tag="p256", name=f"vp{li}{u}{f}")
                    for k in range(2):
                        nc.tensor.matmul(ps[:], src[:, k, f * 128:(f + 1) * 128],
                                         wv_t[:, k, :], start=(k == 0), stop=(k == 1))
                    tv = vt[u][f][:].rearrange("p (h c) -> p h c", h=4)
                    nc.vector.tensor_copy(_r(tv[:, :, 0:64]),
                                          ps[:].rearrange("p (h c) -> p h c", c=64))

            def attn_unit(u):
                msgt[u] = a2.tile([128, 2, NL], F32, tag=f"m{u}", name=f"m{li}{u}")
                for h in range(H):
                    kt_t = kt[u]
                    hc = h // 2
                    r0 = (h % 2) * 64
                    mg = pmsg.tile([65, NL], F32, tag="pmsg", name=f"mg{li}{u}{h}")
                    # two 4-wide score blocks; exp of block a overlaps block
                    # b's matmuls, msg matmuls run once the exp lands
                    exs = []
                    for blk in range(2):
                        sc = p4.tile([128, 4, NL], F32, tag="p4",
                                     name=f"sc{li}{u}{h}{blk}")
                        for j in range(4):
                            f = blk * 4 + j
                            nc.tensor.matmul(
                                sc[:, j, :],
                                _r(kt_t[r0:r0 + 64, hc, f * 128:(f + 1) * 128]),
                                _r(qt[u][r0:r0 + 64, hc, :]),
                                start=True, stop=True)
                        ex = ep.tile([128, 4, NL], F32, tag="ep",
                                     name=f"ex{li}{u}{h}{blk}")
                        nc.scalar.activation(_r(ex[:]), sc[:], AF.Exp)
                        exs.append(ex)
                    for blk in range(2):
                        for j in range(4):
                            f = blk * 4 + j
                            nc.tensor.matmul(mg[:],
                                             _r(vt[u][f][:, h * 65:(h + 1) * 65]),
                                             _r(exs[blk][:, j, :]),
                                             start=(f == 0), stop=(f == 7))
                    rec = a2.tile([1, NL], F32, tag="rec", name=f"rec{li}{u}{h}")
                    nc.vector.reciprocal(rec[:], mg[64:65, :])
                    rbc = a2.tile([64, NL], F32, tag="rbc", name=f"rbc{li}{u}{h}")
                    nc.gpsimd.partition_broadcast(rbc[:], rec[:])
                    nc.vector.tensor_tensor(_r(msgt[u][r0:r0 + 64, hc, :]),
                                            mg[0:64, :], rbc[:], OP.mult)

            def _stats_exchange(us):
                tg = "".join(str(u) for u in us)
                nu = len(us)
                bni = dp.tile([128, nu, 2, 4], F32, tag=f"bni{tg}",
                              name=f"bni{li}{tg}")
                bno = dp.tile([8, 128, nu, 2, 4], F32, tag=f"bno{tg}",
                              name=f"bno{li}{tg}")
                for j, u in enumerate(us):
                    nc.gpsimd.dma_start(out=bni[:, j], in_=stl_d[u][:])
                if use_coll:
                    nc.gpsimd.collective_compute("AllGather", OP.bypass,
                                                 replica_groups=RG_ALL,
                                                 ins=[bni[:].opt()],
                                                 outs=[bno[:].opt()])
                else:
                    nc.sync.dma_start(out=bno[0], in_=bni[:])
                sg8 = a2.tile([128, 8, nu, 2, 4], F32, tag=f"sg8{tg}",
                              name=f"sg8{li}{tg}")
                nc.gpsimd.dma_start(
                    out=sg8[:], in_=bno[:].rearrange("g p u s m -> p g u s m"))
                s4 = a2.tile([128, 4, nu, 2, 4], F32, tag=f"s4{tg}",
                             name=f"s4{li}{tg}")
                nc.gpsimd.tensor_add(s4[:], sg8[:, 0:4], sg8[:, 4:8])
                s2 = a2.tile([128, 2, nu, 2, 4], F32, tag=f"s2{tg}",
                             name=f"s2{li}{tg}")
                nc.gpsimd.tensor_add(s2[:], s4[:, 0:2], s4[:, 2:4])
                sg = a2.tile([128, nu, 2, 4], F32, tag=f"stg{tg}",
                             name=f"stg{li}{tg}")
                nc.gpsimd.tensor_add(sg[:], s2[:, 0], s2[:, 1])
                for j, u in enumerate(us):
                    stg[u] = sg[:, j]
                sc_t, sh_t = bn_scalars(sg[:], us, li, g1_c, be1_c)
                for j, u in enumerate(us):
                    scsh[u] = (sc_t[:, j], sh_t[:, j])

            def mlp_unit(u):
                msgc = a2.tile([128, 2, NL], F32, tag=f"mc{u}", name=f"mc{li}{u}")
                for mo in range(2):
                    ps = p256.tile([128, NL], F32, tag="p256", name=f"cp{li}{u}{mo}")
                    for k in range(2):
                        nc.tensor.matmul(ps[:],
                                         _r(wm_t[:, k, mo * 128:(mo + 1) * 128]),
                                         _r(msgt[u][:, k, :]),
                                         start=(k == 0), stop=(k == 1))
                    nc.vector.tensor_scalar(_r(msgc[:, mo, :]), ps[:],
                                            bm_c[:, mo:mo + 1], None, OP.add)
                ych = [xcur[:, u, 0, :], xcur[:, u, 1, :],
                       msgc[:, 0, :], msgc[:, 1, :]]
                stl = a2.tile([128, 2, 4], F32, tag=f"stl{u}", name=f"stl{li}{u}")
                htile = a1.tile([128, 4, NL], F32, tag=f"h{u}", name=f"h{li}{u}")
                for mo in range(4):
                    ps = p256.tile([128, NL], F32, tag="p256", name=f"h1p{li}{u}{mo}")
                    for k in range(4):
                        nc.tensor.matmul(ps[:],
                                         _r(w1_t[:, k, mo * 128:(mo + 1) * 128]),
                                         _r(ych[k]), start=(k == 0), stop=(k == 3))
                    nc.scalar.activation(htile[:, mo, :], ps[:], AF.Identity,
                                         bias=b1_c[:, mo:mo + 1],
                                         accum_out=stl[:, 0, mo:mo + 1])
                    sqs = ep.tile([128, NL], F32, tag="sq", name=f"sq{li}{u}{mo}")
                    nc.scalar.activation(sqs[:], htile[:, mo, :], AF.Square,
                                         accum_out=stl[:, 1, mo:mo + 1])
                ht[u] = htile
                stl_d[u] = stl
                if not merge_stats:
                    _stats_exchange((u,))

            def make_cx(u):
                return dict(u=u, li=li, i=i, npar=npar, xn=xn, xn16=xn16,
                            sc=scsh[u][0], sh=scsh[u][1], ht=ht[u],
                            resid=resid, w2=w2_t, b2=b2_c,
                            last=(li == n_layers - 1))

            kv_unit(uA, slabs[sA])
            flush_pending()
            q_unit(uA)
            q_unit(uB)
            attn_unit(uA)
            mlp_unit(uA)
            kv_unit(uB, slabs[sB])
            def _tail_A():
                cxA = make_cx(uA)
                bn_conv2(cxA)
                if li < n_layers - 1:
                    gather(cxA)
            if merge_stats:
                attn_unit(uB)
                mlp_unit(uB)
                _stats_exchange((0, 1))
                _tail_A()
            elif bnA_pos == "after_kvB":
                _tail_A()
                attn_unit(uB)
                mlp_unit(uB)
            elif bnA_pos == "after_attnB":
                attn_unit(uB)
                _tail_A()
                mlp_unit(uB)
            else:  # after_mlpB
                attn_unit(uB)
                mlp_unit(uB)
                _tail_A()
            state["pending"] = make_cx(uB)
            state["xprev"] = xn
            if not defer:
                flush_pending()

        flush_pending()
        _es.close()

    nc.finalize()
    return nc


def _host_prep(inputs):
    f = np.float32
    Wq, bq = np.asarray(inputs["Wq"], f), np.asarray(inputs["bq"], f)
    Wk = np.asarray(inputs["Wk"], f)
    Wv, bv = np.asarray(inputs["Wv"], f), np.asarray(inputs["bv"], f)
    Wm, bm = np.asarray(inputs["Wm"], f), np.asarray(inputs["bm"], f)
    W1, b1 = np.asarray(inputs["W1"], f), np.asarray(inputs["b1"], f)
    g1, be1 = np.asarray(inputs["g1"], f), np.asarray(inputs["be1"], f)
    W2, b2 = np.asarray(inputs["W2"], f), np.asarray(inputs["b2"], f)
    d0, d1 = np.asarray(inputs["desc0"], f), np.asarray(inputs["desc1"], f)

    SCALE = f(1.0 / np.sqrt(HD))

    def lhsT(w, kc=2):
        # w: [L, out, in] -> partition-major lhsT [L, 128, kc, out]
        t = w.transpose(0, 2, 1).reshape(L, kc, 128, w.shape[1])
        return np.ascontiguousarray(t.transpose(0, 2, 1, 3))

    wqt = lhsT(Wq[:, PERM, :] * SCALE)
    wkt = lhsT(Wk[:, PERM, :])
    wvt = lhsT(Wv[:, PERM, :])            # rhs [in-chunks, out_perm] — same form
    wmt = lhsT(Wm[:, :, PERM])
    wqm = np.ascontiguousarray(np.stack([wqt, wmt], axis=3))
    wkv = np.ascontiguousarray(
        np.stack([wkt, wvt], axis=3).astype(NP_BF16))
    w1t = lhsT(W1, kc=4)
    w2t = lhsT(W2, kc=4)

    bq_a = (bq[:, PERM] * SCALE).reshape(L, 2, 128).transpose(0, 2, 1)
    bm_eff = (np.einsum("loi,li->lo", Wm, bv) + bm).astype(f)
    bm_a = bm_eff.reshape(L, 2, 128).transpose(0, 2, 1)
    b1_a = b1.reshape(L, 4, 128).transpose(0, 2, 1)
    b2_a = b2.reshape(L, 2, 128).transpose(0, 2, 1)
    g1_a = g1.reshape(L, 4, 128).transpose(0, 2, 1)
    be1_a = be1.reshape(L, 4, 128).transpose(0, 2, 1)
    bia = np.concatenate([bq_a, bm_a, b1_a, b2_a, g1_a, g1_a, be1_a, be1_a], axis=2)
    bia = np.ascontiguousarray(bia.astype(f))
    assert bia.shape == (L, 128, 26)

    shared = dict(wqm=wqm, wkv=wkv, w1t=w1t, w2t=w2t, bia=bia)
    in_maps = []
    for c in range(8):
        b, q = c // 4, c % 4
        m = dict(shared)
        dsc = np.stack([d0[b].reshape(2, 128, N), d1[b].reshape(2, 128, N)], axis=2)
        m["dsc"] = np.ascontiguousarray(dsc.transpose(1, 0, 2, 3)).astype(NP_BF16)
        dlo = np.stack([d0[b][:, q * NL:(q + 1) * NL].reshape(2, 128, NL),
                        d1[b][:, q * NL:(q + 1) * NL].reshape(2, 128, NL)], axis=0)
        m["dlo"] = np.ascontiguousarray(dlo.transpose(2, 0, 1, 3))
        in_maps.append(m)
    return in_maps, d0, d1


class _Exec:
    """Compiled 8-core executable: jit cached across kernel() calls."""

    def __init__(self, n_layers=L):
        import jax
        from jax.experimental.shard_map import shard_map
        from jax.sharding import Mesh, PartitionSpec

        nc = _build_program(n_layers=n_layers)
        bass2jax.install_neuronx_cc_hook()
        partition_name = (nc.partition_id_tensor.name
                          if nc.partition_id_tensor else None)
        in_names, out_names, out_avals = [], [], []
        for alloc in nc.m.functions[0].allocations:
            if not isinstance(alloc, mybir.MemoryLocationSet):
                continue
            name = alloc.memorylocations[0].name
            if alloc.kind == "ExternalInput":
                if name != partition_name:
                    in_names.append(name)
            elif alloc.kind == "ExternalOutput":
                assert alloc.tensor_shape is not None and alloc.dtype is not None
                out_names.append(name)
                out_avals.append(jax.core.ShapedArray(
                    tuple(alloc.tensor_shape), mybir.dt.np(alloc.dtype)))
        assert nc.dbg_addr is None
        n_params = len(in_names)
        all_names = tuple(in_names) + tuple(out_names)
        if partition_name is not None:
            all_names = all_names + (partition_name,)
        out_avals_t = tuple(out_avals)
        out_names_t = tuple(out_names)

        def _body(*args):
            operands = list(args)
            if partition_name is not None:
                operands.append(bass2jax.partition_id_tensor())
            outs = bass2jax._bass_exec_p.bind(
                *operands,
                out_avals=out_avals_t,
                in_names=all_names,
                out_names=out_names_t,
                lowering_input_output_aliases=(),
                sim_require_finite=True,
                sim_require_nnan=True,
                nc=nc,
            )
            return tuple(outs)

        devices = jax.devices()[:8]
        assert len(devices) == 8
        self.mesh = Mesh(np.asarray(devices), ("core",))
        self.pspec = PartitionSpec("core")
        n_args = n_params + len(out_names)
        donate = tuple(range(n_params, n_args))
        self.sharded = jax.jit(
            shard_map(_body, mesh=self.mesh,
                      in_specs=(self.pspec,) * n_args,
                      out_specs=(self.pspec,) * len(out_names),
                      check_rep=False),
            donate_argnums=donate, keep_unused=True)
        self.nc = nc
        self.in_names = in_names
        self.out_names = out_names
        self.out_avals = out_avals
        self.n_params = n_params

    def concat_inputs(self, in_maps):
        return [np.concatenate([np.asarray(m[name]) for m in in_maps], axis=0)
                for name in self.in_names]

    def make_zeros_host(self):
        return [np.zeros((8 * a.shape[0], *a.shape[1:]), a.dtype)
                for a in self.out_avals]

    def run(self, concat_in, zeros):
        import jax
        outs = self.sharded(*concat_in, *zeros)
        jax.block_until_ready(outs)
        return outs

    def split_outputs(self, outs):
        res = []
        for c in range(8):
            m = {}
            for i, name in enumerate(self.out_names):
                a = np.asarray(outs[i])
                m[name] = a.reshape(8, *self.out_avals[i].shape)[c]
            res.append(m)
        return res


def _get_exec(n_layers=L):
    key = f"exec{n_layers}"
    if key not in _CACHE:
        _CACHE[key] = _Exec(n_layers=n_layers)
    return _CACHE[key]


def kernel(**inputs):
    ex = _get_exec()
    in_maps, d0, d1 = _host_prep(inputs)
    concat_in = ex.concat_inputs(in_maps)
    outs = ex.run(concat_in, ex.make_zeros_host())
    results = ex.split_outputs(outs)

    full = [np.zeros((B, D, N), np.float32) for _ in range(2 * L + 2)]
    full[2] = d0.copy(); full[3] = d1.copy()
    for c in range(8):
        b, q = c // 4, c % 4
        O = results[c]["out"]  # [L, 128, 2, 2, NL]
        for i in range(L):
            for u in range(2):
                j = u if i == 0 else (4 + u if i == 1 else 2 * i + 2 + u)
                full[j][b, :, q * NL:(q + 1) * NL] = \
                    O[i, :, u].transpose(1, 0, 2).reshape(D, NL)
    return tuple(full)
